# revision 16
# baseline (speedup 1.0000x reference)
"""Trainium2 Bass kernel for nn_Decoder (gnn_message_passing).

Data-parallel over batch N=64 across 8 NeuronCores (8 samples/core).

Per-sample layout: channels c on partitions (2 chunks of 128), l = (t, v)
with v innermost, v permuted part-packed:
  PERM = [1,2,3,4, 5,6,7,8, 0,9,10,11,12, 13..16, 17..20]
so each body part is one contiguous v-range. The adjacency A is permuted
on the host to match; the final output is un-permuted on the host.

Pipeline per sample (all heavy matmuls in fp32r = s8e11m rounded operands,
fp32 accumulate):
  A) AdaIN: bn_stats for x/s parts, tiny MLP (Prelu alpha=0.2), fused
     affine+leaky-relu apply -> x1 (c, l) f32r
  B) out1T (l-chunk 126, 768) = x1^T @ Wall^T   [stationary = x1 chunks]
  C) out2T (l-chunk, c) = sum_k Bd_k^T-style matmul [stationary = kron(I6, A_k)]
     then PE-transpose back to (c, l-chunk), + folded gcn bias -> out2p (padded t)
  D) tcn: 36 matmuls over shifted windows of out2p, + tcn bias -> x2 (c, l)
  E) astyle per part: inorm (fused in ACT), F/G convs, HmT via stationary=s,
     S_pre^T via stationary=Gm, exp (no max-sub needed; values bounded),
     softmax denominator via ones-matmul, O = HmT.T @ E^T, wk conv,
     deferred 1/D scaling, +bk' (bh folded via softmax-sum=1), residual -> x3
  F) = B/C/D with layer-2 weights -> y
"""
import numpy as np

import concourse.bacc as bacc
import concourse.tile as tile
from concourse import mybir
from concourse.bass_utils import run_bass_kernel_spmd
from concourse.masks import make_identity

F32 = mybir.dt.float32
F32R = mybir.dt.float32r

N, C, T, V, K, KT, LAT = 64, 256, 60, 21, 3, 3, 64
IDX = [[1, 2, 3, 4], [5, 6, 7, 8], [0, 9, 10, 11, 12], [13, 14, 15, 16], [17, 18, 19, 20]]
PERM = np.array(sum(IDX, []))
PART_OFF = [0, 4, 8, 13, 17, 21]
NPARTS = 5
NCORES = 8
L = T * V               # 1260
NL = 10                 # l-chunks
LC = L // NL            # 126 = 6 t-rows
TG = 6                  # t-rows per l-chunk
ALPHA = 0.2
EPS = 1e-5

# l2-chunk sizes per part (aligned to whole t-rows)
PART_LP = [PART_OFF[i + 1] - PART_OFF[i] for i in range(NPARTS)]        # joints
LP = [T * v for v in PART_LP]                                            # 240/300
LPP = [max(256, lp) for lp in LP]                                        # padded N
SOFF = [60 * o for o in PART_OFF]
M_CHUNKS = {240: [(0, 120), (120, 240)], 300: [(0, 100), (100, 200), (200, 300)]}


def build_kernel(ns):
    """Build the Bass program for `ns` samples per core. Returns nc."""
    nc = bacc.Bacc("TRN2", target_bir_lowering=False)

    # ---------------- DRAM tensors ----------------
    d_x = nc.dram_tensor("x", [ns, 2, 128, T, V], F32, kind="ExternalInput").ap()
    d_s = nc.dram_tensor("s", [ns, 2, 128, L], F32R, kind="ExternalInput").ap()
    d_wall = [nc.dram_tensor(f"wall{ly}", [2, 128, K * C], F32R, kind="ExternalInput").ap()
              for ly in range(2)]
    d_bd = nc.dram_tensor("bd", [K, LC, LC], F32R, kind="ExternalInput").ap()
    d_tcnw = [nc.dram_tensor(f"tcnw{ly}", [KT, 2, 128, C], F32R, kind="ExternalInput").ap()
              for ly in range(2)]
    d_b2v = [nc.dram_tensor(f"b2v{ly}", [2, 128, 1, V], F32, kind="ExternalInput").ap()
             for ly in range(2)]
    d_tcnb = [nc.dram_tensor(f"tcnb{ly}", [2, 128, 1], F32, kind="ExternalInput").ap()
              for ly in range(2)]
    d_wlT = nc.dram_tensor("wlT", [NPARTS, 2, 128, LAT], F32, kind="ExternalInput").ap()
    d_w1T = nc.dram_tensor("w1T", [NPARTS, LAT, LAT], F32, kind="ExternalInput").ap()
    d_w2T = nc.dram_tensor("w2T", [NPARTS, LAT, 2 * C], F32, kind="ExternalInput").ap()
    d_blT = nc.dram_tensor("blT", [LAT, NPARTS], F32, kind="ExternalInput").ap()
    d_b1T = nc.dram_tensor("b1T", [LAT, NPARTS], F32, kind="ExternalInput").ap()
    d_b2q = nc.dram_tensor("b2q", [128, NPARTS, 4], F32, kind="ExternalInput").ap()
    d_astw = {nm: nc.dram_tensor(f"ast_w{nm}", [NPARTS, 2, 128, C], F32R,
                                 kind="ExternalInput").ap() for nm in "fghk"}
    d_bf = nc.dram_tensor("bfT", [2, 128, NPARTS], F32, kind="ExternalInput").ap()
    d_bg = nc.dram_tensor("bgT", [2, 128, NPARTS], F32, kind="ExternalInput").ap()
    d_bkp = nc.dram_tensor("bkpT", [2, 128, NPARTS], F32, kind="ExternalInput").ap()
    d_invLp = nc.dram_tensor("invLp", [128, NPARTS], F32, kind="ExternalInput").ap()
    d_ones = nc.dram_tensor("ones", [128, 1], F32R, kind="ExternalInput").ap()
    d_onesT = nc.dram_tensor("onesT", [1, 128], F32R, kind="ExternalInput").ap()
    d_y = nc.dram_tensor("y", [ns, 2, 128, T, V], F32, kind="ExternalOutput").ap()

    AF = mybir.ActivationFunctionType
    OP = mybir.AluOpType

    with tile.TileContext(nc) as tc:
        with tc.tile_pool(name="consts", bufs=1) as cs, \
             tc.tile_pool(name="stream", bufs=1) as st, \
             tc.tile_pool(name="psum", bufs=8, space="PSUM") as ps:
            _ctr = [0]

            def _mk(pool):
                def f(shape, dtype=F32, tag="t", bufs=None):
                    _ctr[0] += 1
                    kw = {} if bufs is None else {"bufs": bufs}
                    return pool.tile(shape, dtype, tag=tag, name=f"{tag}_{_ctr[0]}", **kw)
                return f
            _cs, _st, _ps = _mk(cs), _mk(st), _mk(ps)
            # ---------------- load constants ----------------
            wall = [[_cs([128, K * C], F32R, tag=f"wall{ly}_{ch}") for ch in range(2)]
                    for ly in range(2)]
            for ly in range(2):
                for ch in range(2):
                    nc.sync.dma_start(out=wall[ly][ch], in_=d_wall[ly][ch])
            bd = [_cs([LC, LC], F32R, tag=f"bd{k}") for k in range(K)]
            for k in range(K):
                nc.sync.dma_start(out=bd[k], in_=d_bd[k])
            tcnw = [[[_cs([128, C], F32R, tag=f"tcnw{ly}_{dt}_{ch}") for ch in range(2)]
                     for dt in range(KT)] for ly in range(2)]
            for ly in range(2):
                for dt in range(KT):
                    for ch in range(2):
                        nc.sync.dma_start(out=tcnw[ly][dt][ch], in_=d_tcnw[ly][dt][ch])
            b2v = [[_cs([128, 1, V], F32, tag=f"b2v{ly}_{ch}") for ch in range(2)]
                   for ly in range(2)]
            tcnb = [[_cs([128, 1], F32, tag=f"tcnb{ly}_{ch}") for ch in range(2)]
                    for ly in range(2)]
            for ly in range(2):
                for ch in range(2):
                    nc.sync.dma_start(out=b2v[ly][ch], in_=d_b2v[ly][ch])
                    nc.sync.dma_start(out=tcnb[ly][ch], in_=d_tcnb[ly][ch])
            wlT = [[_cs([128, LAT], F32, tag=f"wlT{i}_{ch}") for ch in range(2)]
                   for i in range(NPARTS)]
            w1T = [_cs([LAT, LAT], F32, tag=f"w1T{i}") for i in range(NPARTS)]
            w2T = [_cs([LAT, 2 * C], F32, tag=f"w2T{i}") for i in range(NPARTS)]
            for i in range(NPARTS):
                for ch in range(2):
                    nc.sync.dma_start(out=wlT[i][ch], in_=d_wlT[i][ch])
                nc.sync.dma_start(out=w1T[i], in_=d_w1T[i])
                nc.sync.dma_start(out=w2T[i], in_=d_w2T[i])
            blT = _cs([LAT, NPARTS], F32, tag="blT")
            b1T = _cs([LAT, NPARTS], F32, tag="b1T")
            b2q = _cs([128, NPARTS, 4], F32, tag="b2q")
            nc.sync.dma_start(out=blT, in_=d_blT)
            nc.sync.dma_start(out=b1T, in_=d_b1T)
            nc.sync.dma_start(out=b2q, in_=d_b2q)
            astw = {nm: [[_cs([128, C], F32R, tag=f"astw{nm}{i}_{ch}")
                          for ch in range(2)] for i in range(NPARTS)] for nm in "fghk"}
            for nm in "fghk":
                for i in range(NPARTS):
                    for ch in range(2):
                        nc.sync.dma_start(out=astw[nm][i][ch], in_=d_astw[nm][i][ch])
            bf = [_cs([128, NPARTS], F32, tag=f"bf{ch}") for ch in range(2)]
            bg = [_cs([128, NPARTS], F32, tag=f"bg{ch}") for ch in range(2)]
            bkp = [_cs([128, NPARTS], F32, tag=f"bkp{ch}") for ch in range(2)]
            for ch in range(2):
                nc.sync.dma_start(out=bf[ch], in_=d_bf[ch])
                nc.sync.dma_start(out=bg[ch], in_=d_bg[ch])
                nc.sync.dma_start(out=bkp[ch], in_=d_bkp[ch])
            ident = _cs([128, 128], F32, tag="ident")
            make_identity(nc, ident)
            onesr = _cs([128, 1], F32R, tag="onesr")
            nc.sync.dma_start(out=onesr, in_=d_ones)
            onesrT = _cs([1, 128], F32R, tag="onesrT")
            nc.sync.dma_start(out=onesrT, in_=d_onesT)
            epsT = _cs([128, 1], F32, tag="epsT")
            nc.vector.memset(epsT, EPS)
            invLp = _cs([128, NPARTS], F32, tag="invLp")
            nc.sync.dma_start(out=invLp, in_=d_invLp)

            def part_stats(get_ap, mtag):
                mean = [_st([128, NPARTS], F32, tag=f"{mtag}m_{ch}", bufs=2)
                        for ch in range(2)]
                var = [_st([128, NPARTS], F32, tag=f"{mtag}v_{ch}", bufs=2)
                       for ch in range(2)]
                for ch in range(2):
                    sums = _st([128, NPARTS], F32, tag="stsum")
                    sqs = _st([128, NPARTS], F32, tag="stsq")
                    for i in range(NPARTS):
                        lo, hi = PART_OFF[i], PART_OFF[i + 1]
                        src_ap = get_ap(ch, i)
                        ax = (mybir.AxisListType.X if len(src_ap.shape) == 2
                              else mybir.AxisListType.XY)
                        nc.vector.reduce_sum(out=sums[:, i:i + 1], in_=src_ap,
                                             axis=ax)
                        sc_ = _st([128, T * 5], F32, tag="sqscr", bufs=2)
                        sc_ap = sc_[:, :LP[i]]
                        if len(src_ap.shape) == 3:
                            sc_ap = sc_ap.rearrange("p (a b) -> p a b",
                                                    b=src_ap.shape[2])
                        nc.scalar.activation(out=sc_ap, in_=src_ap,
                                             func=AF.Square,
                                             accum_out=sqs[:, i:i + 1])
                    nc.vector.tensor_mul(mean[ch], sums, invLp)
                    msq = _st([128, NPARTS], F32, tag="stmsq")
                    nc.vector.tensor_mul(msq, sqs, invLp)
                    m2 = _st([128, NPARTS], F32, tag="stm2")
                    nc.vector.tensor_mul(m2, mean[ch], mean[ch])
                    nc.vector.tensor_sub(var[ch], msq, m2)
                return mean, var

            def mm(out, pairs, **kw):
                nmm = len(pairs)
                for j, (lh, rh) in enumerate(pairs):
                    nc.tensor.matmul(out, lh, rh, start=(j == 0), stop=(j == nmm - 1), **kw)

            # ---------------- gcn + tcn block ----------------
            def gcn_tcn(ly, xin, xout_dtype, bias_out_tag):
                """xin: [2] tiles (128, T, V) f32r. Returns x2 [2] tiles (128,T,V)."""
                out2p = [_st([128, T + 2, V], F32R, tag=f"out2p_{ch}") for ch in range(2)]
                for g in range(NL):
                    psB = [_ps([LC, 384], F32, tag="mm") for _ in range(2)]
                    for nsl in range(2):
                        mm(psB[nsl],
                           [(xin[ch][:, g * TG:(g + 1) * TG, :],
                             wall[ly][ch][:, nsl * 384:(nsl + 1) * 384]) for ch in range(2)])
                    o1 = _st([LC, K * C], F32R, tag="out1T", bufs=2)
                    for nsl in range(2):
                        nc.vector.tensor_copy(o1[:, nsl * 384:(nsl + 1) * 384], psB[nsl])
                    psC = _ps([LC, C], F32, tag="mm")
                    mm(psC, [(bd[k], o1[:, k * C:(k + 1) * C]) for k in range(K)])
                    o2 = _st([LC, C], F32, tag="out2T", bufs=2)
                    nc.vector.tensor_copy(o2, psC)
                    for ch in range(2):
                        psT = _ps([128, LC], F32, tag="mm")
                        nc.tensor.transpose(psT, o2[:, ch * 128:(ch + 1) * 128],
                                            ident[:LC, :LC])
                        # evict + folded gcn bias -> padded rows [1..61)
                        nc.vector.tensor_tensor(
                            out=out2p[ch][:, g * TG + 1:(g + 1) * TG + 1, :],
                            in0=psT.rearrange("p (a b) -> p a b", b=V),
                            in1=b2v[ly][ch].to_broadcast([128, TG, V]),
                            op=OP.add)
                # reflect pads: t'=0 <- t'=2 ; t'=61 <- t'=59
                for ch in range(2):
                    nc.vector.tensor_copy(out2p[ch][:, 0, :], out2p[ch][:, 2, :])
                    nc.vector.tensor_copy(out2p[ch][:, T + 1, :], out2p[ch][:, T - 1, :])
                # tcn
                x2 = [_st([128, T, V], xout_dtype, tag=f"{bias_out_tag}_{ch}",
                           bufs=(2 if bias_out_tag == "y" else 1))
                      for ch in range(2)]
                for cch in range(2):
                    flat = [out2p[ch].rearrange("p a b -> p (a b)") for ch in range(2)]
                    for nsl in range(3):
                        psD = _ps([128, 420], F32, tag="mm")
                        mm(psD, [(tcnw[ly][dt][cin][:, cch * 128:(cch + 1) * 128],
                                  flat[cin][:, dt * V + nsl * 420:dt * V + nsl * 420 + 420])
                                 for cin in range(2) for dt in range(KT)])
                        nc.scalar.activation(
                            out=x2[cch].rearrange("p a b -> p (a b)")[:, nsl * 420:(nsl + 1) * 420],
                            in_=psD, func=AF.Identity, bias=tcnb[ly][cch], scale=1.0)
                return x2

            # ---------------- per-sample ----------------
            for n in range(ns):
                x_sb = [_st([128, T, V], F32, tag=f"x_{ch}") for ch in range(2)]
                s_sb = [_st([128, L], F32R, tag=f"s_{ch}") for ch in range(2)]
                for ch in range(2):
                    nc.sync.dma_start(out=x_sb[ch], in_=d_x[n, ch])
                    nc.sync.dma_start(out=s_sb[ch], in_=d_s[n, ch])

                # ---- Phase A: stats ----
                xmean, xvar = part_stats(
                    lambda ch, i: x_sb[ch][:, :, PART_OFF[i]:PART_OFF[i + 1]], "mvx")
                smean, svar = part_stats(
                    lambda ch, i: s_sb[ch][:, SOFF[i]:SOFF[i + 1]].bitcast(F32), "mvs")
                sbar = [_st([128, NPARTS], F32, tag=f"sbar_{ch}") for ch in range(2)]
                for ch in range(2):
                    nc.vector.tensor_copy(sbar[ch], smean[ch])

                # ---- Phase A: MLP ----
                gbe = _st([128, NPARTS, 4], F32, tag="gbe")
                for i in range(NPARTS):
                    psZ = _ps([LAT, 1], F32, tag="mm")
                    mm(psZ, [(wlT[i][ch], sbar[ch][:, i:i + 1]) for ch in range(2)])
                    z = _st([LAT, 1], F32, tag="z")
                    nc.scalar.activation(out=z, in_=psZ, func=AF.Prelu,
                                         bias=blT[:, i:i + 1], scale=1.0, alpha=ALPHA)
                    psH = _ps([LAT, 1], F32, tag="mm")
                    mm(psH, [(w1T[i], z)])
                    h = _st([LAT, 1], F32, tag="z")
                    nc.scalar.activation(out=h, in_=psH, func=AF.Prelu,
                                         bias=b1T[:, i:i + 1], scale=1.0, alpha=ALPHA)
                    psG = _ps([128, 4], F32, tag="mm")
                    for j in range(4):
                        nc.tensor.matmul(psG[:, j:j + 1], w2T[i][:, j * 128:(j + 1) * 128],
                                         h, start=True, stop=True)
                    nc.vector.tensor_tensor(out=gbe[:, i, :], in0=psG, in1=b2q[:, i, :],
                                            op=OP.add)

                # ---- Phase A: massage + apply ----
                x1 = [_st([128, T, V], F32R, tag=f"x1_{ch}") for ch in range(2)]
                for ch in range(2):
                    sq = _st([128, NPARTS], F32, tag="sq")
                    nc.scalar.activation(out=sq, in_=xvar[ch], func=AF.Sqrt,
                                         bias=epsT, scale=1.0)
                    rstd = _st([128, NPARTS], F32, tag="rstd")
                    nc.vector.reciprocal(out=rstd, in_=sq)
                    scale = _st([128, NPARTS], F32, tag="scl")
                    nc.vector.scalar_tensor_tensor(out=scale, in0=gbe[:, :, ch],
                                                   scalar=1.0, in1=rstd,
                                                   op0=OP.add, op1=OP.mult)
                    tmp = _st([128, NPARTS], F32, tag="tmp5")
                    nc.vector.tensor_mul(tmp, xmean[ch], scale)
                    shift = _st([128, NPARTS], F32, tag="shf")
                    nc.vector.scalar_tensor_tensor(out=shift, in0=tmp, scalar=-1.0,
                                                   in1=gbe[:, :, 2 + ch],
                                                   op0=OP.mult, op1=OP.add)
                    for i in range(NPARTS):
                        lo, hi = PART_OFF[i], PART_OFF[i + 1]
                        nc.scalar.activation(out=x1[ch][:, :, lo:hi],
                                             in_=x_sb[ch][:, :, lo:hi], func=AF.Prelu,
                                             bias=shift[:, i:i + 1],
                                             scale=scale[:, i:i + 1], alpha=ALPHA)

                # ---- Phase B+C+D: layer 1 ----
                x2 = gcn_tcn(0, x1, F32, "x2")

                # ---- Phase E: astyle ----
                # x2 part stats
                mean2, var2 = part_stats(
                    lambda ch, i: x2[ch][:, :, PART_OFF[i]:PART_OFF[i + 1]], "mv2")
                # massage: rstd/shift for x2 and s
                rs2, sh2, rss, shs = [], [], [], []
                for ch in range(2):
                    for (mn, vr, rs_l, sh_l, t_) in (
                            (mean2[ch], var2[ch], rs2, sh2, "i2"),
                            (smean[ch], svar[ch], rss, shs, "is")):
                        sq = _st([128, NPARTS], F32, tag="sq")
                        nc.scalar.activation(out=sq, in_=vr, func=AF.Sqrt,
                                             bias=epsT, scale=1.0)
                        rstd = _st([128, NPARTS], F32, tag=f"rstd_{t_}", bufs=2)
                        nc.vector.reciprocal(out=rstd, in_=sq)
                        shift = _st([128, NPARTS], F32, tag=f"shf_{t_}", bufs=2)
                        tmp = _st([128, NPARTS], F32, tag="tmp5")
                        nc.vector.tensor_mul(tmp, mn, rstd)
                        nc.vector.tensor_scalar_mul(shift, tmp, -1.0)
                        rs_l.append(rstd)
                        sh_l.append(shift)

                x3 = [_st([128, T, V], F32R, tag=f"x3_{ch}") for ch in range(2)]
                for i in range(NPARTS):
                    lo, hi = PART_OFF[i], PART_OFF[i + 1]
                    lp, lpp = LP[i], LPP[i]
                    mchunks = M_CHUNKS[lp]
                    inx = [_st([128, lpp], F32R, tag=f"inx_{ch}") for ch in range(2)]
                    ins = [_st([128, lpp], F32R, tag=f"ins_{ch}") for ch in range(2)]
                    for ch in range(2):
                        nc.scalar.activation(
                            out=inx[ch][:, :lp].rearrange("p (a b) -> p a b", b=T),
                            in_=x2[ch][:, :, lo:hi].rearrange("p a b -> p b a"),
                            func=AF.Identity,
                            bias=sh2[ch][:, i:i + 1], scale=rs2[ch][:, i:i + 1])
                        nc.scalar.activation(
                            out=ins[ch][:, :lp],
                            in_=s_sb[ch][:, SOFF[i]:SOFF[i + 1]].bitcast(F32),
                            func=AF.Identity,
                            bias=shs[ch][:, i:i + 1], scale=rss[ch][:, i:i + 1])
                        if lp < lpp:
                            for padt in (inx[ch], ins[ch]):
                                nc.scalar.activation(
                                    out=padt[:, lp:], in_=ident[:, :lpp - lp],
                                    func=AF.Identity, scale=0.0, bias=0.0)
                    Fm, Gm = [], []
                    for (wt, src, bias_t, dst) in (("f", inx, bf, Fm), ("g", ins, bg, Gm)):
                        for cout in range(2):
                            psF = _ps([128, lpp], F32, tag="mm")
                            mm(psF, [(astw[wt][i][cin][:, cout * 128:(cout + 1) * 128],
                                      src[cin]) for cin in range(2)])
                            o = _st([128, lpp], F32R, tag=f"{wt}m_{cout}")
                            nc.scalar.activation(out=o, in_=psF, func=AF.Identity,
                                                 bias=bias_t[cout][:, i:i + 1], scale=1.0)
                            dst.append(o)
                    HmT = []
                    for (mlo, mhi) in mchunks:
                        sz = mhi - mlo
                        psHm = _ps([sz, C], F32, tag="mm")
                        mm(psHm, [(s_sb[cin][:, SOFF[i] + mlo:SOFF[i] + mhi],
                                   astw["h"][i][cin]) for cin in range(2)])
                        o = _st([128, C], F32R, tag="HmT", bufs=3)
                        nc.vector.tensor_copy(o[:sz], psHm)
                        HmT.append(o)
                    Et = []
                    for mi_, (mlo, mhi) in enumerate(mchunks):
                        sz = mhi - mlo
                        psS = _ps([sz, lpp], F32, tag="mm")
                        mm(psS, [(Gm[cch][:, mlo:mhi], Fm[cch]) for cch in range(2)])
                        o = _st([128, lpp], F32R, tag="Et", bufs=3)
                        nc.scalar.activation(out=o[:sz], in_=psS, func=AF.Exp, scale=1.0)
                        Et.append(o)
                    psDs = _ps([1, lp], F32, tag="mm")
                    mm(psDs, [(onesr[:mhi - mlo], Et[j][:mhi - mlo, :lp])
                              for j, (mlo, mhi) in enumerate(mchunks)])
                    rD = _st([1, lp], F32R, tag="rD")
                    with nc.allow_low_precision(reason="softmax denom as f32r matmul operand"):
                        nc.vector.reciprocal(out=rD, in_=psDs)
                    psDb = _ps([128, lp], F32, tag="mm")
                    mm(psDb, [(onesrT, rD)])
                    Db = _st([128, lp], F32, tag="Db")
                    nc.vector.tensor_copy(Db, psDb)
                    O_sb = []
                    for cout in range(2):
                        psO = _ps([128, lpp], F32, tag="mm")
                        mm(psO, [(HmT[j][:mhi - mlo, cout * 128:(cout + 1) * 128],
                                  Et[j][:mhi - mlo])
                                 for j, (mlo, mhi) in enumerate(mchunks)])
                        o = _st([128, lpp], F32R, tag=f"O_{cout}")
                        nc.vector.tensor_copy(o, psO)
                        O_sb.append(o)
                    for cout in range(2):
                        psK = _ps([128, lpp], F32, tag="mm")
                        mm(psK, [(astw["k"][i][cin][:, cout * 128:(cout + 1) * 128],
                                  O_sb[cin]) for cin in range(2)])
                        t1_ = _st([128, hi - lo, T], F32, tag="wkt")
                        nc.vector.tensor_tensor(
                            out=t1_,
                            in0=psK[:, :lp].rearrange("p (a b) -> p a b", b=T),
                            in1=Db.rearrange("p (a b) -> p a b", b=T),
                            op=OP.mult)
                        nc.vector.scalar_tensor_tensor(
                            out=x3[cout][:, :, lo:hi].rearrange("p a b -> p b a"),
                            in0=t1_, scalar=bkp[cout][:, i:i + 1],
                            in1=x2[cout][:, :, lo:hi].rearrange("p a b -> p b a"),
                            op0=OP.add, op1=OP.add)

                # ---- Phase F: layer 2 ----
                y_sb = gcn_tcn(1, x3, F32, "y")
                for ch in range(2):
                    nc.sync.dma_start(out=d_y[n, ch], in_=y_sb[ch])

    nc.compile()
    return nc


def prepare_weights(inp):
    """Host-side weight preprocessing -> dict of DRAM arrays (shared across cores)."""
    f32 = np.float32
    Ap = np.ascontiguousarray(inp["A"][:, PERM][:, :, PERM]).astype(f32)
    out = {}
    out["bd"] = np.stack([np.kron(np.eye(TG, dtype=f32), Ap[k]) for k in range(K)])
    for ly, (gw, gb, tw, tb) in enumerate(
            ((inp["gcn1_w"], inp["gcn1_b"], inp["tcn1_w"], inp["tcn1_b"]),
             (inp["gcn2_w"], inp["gcn2_b"], inp["tcn2_w"], inp["tcn2_b"]))):
        out[f"wall{ly}"] = np.ascontiguousarray(gw.T.reshape(2, 128, K * C)).astype(f32)
        out[f"tcnw{ly}"] = np.ascontiguousarray(
            tw[:, :, :, 0].transpose(2, 1, 0).reshape(KT, 2, 128, C)).astype(f32)
        bias2 = np.einsum("kc,kw->cw", gb.reshape(K, C), Ap.sum(axis=1)).astype(f32)
        out[f"b2v{ly}"] = np.ascontiguousarray(
            bias2.reshape(2, 128, 1, V)).astype(f32)
        out[f"tcnb{ly}"] = np.ascontiguousarray(tb.reshape(2, 128, 1)).astype(f32)
    out["wlT"] = np.ascontiguousarray(
        inp["adain_wl"].transpose(0, 2, 1).reshape(NPARTS, 2, 128, LAT)).astype(f32)
    out["w1T"] = np.ascontiguousarray(inp["adain_w1"].transpose(0, 2, 1)).astype(f32)
    out["w2T"] = np.ascontiguousarray(inp["adain_w2"].transpose(0, 2, 1)).astype(f32)
    out["blT"] = np.ascontiguousarray(inp["adain_bl"].T).astype(f32)
    out["b1T"] = np.ascontiguousarray(inp["adain_b1"].T).astype(f32)
    out["b2q"] = np.ascontiguousarray(
        inp["adain_b2"].reshape(NPARTS, 4, 128).transpose(2, 0, 1)).astype(f32)
    for nm in "fgh":
        out[f"ast_w{nm}"] = np.ascontiguousarray(
            inp[f"ast_w{nm}"].transpose(0, 2, 1).reshape(NPARTS, 2, 128, C)).astype(f32)
    out["ast_wk"] = np.ascontiguousarray(
        inp["ast_wk"].transpose(0, 2, 1).reshape(NPARTS, 2, 128, C)).astype(f32)
    out["bfT"] = np.ascontiguousarray(inp["ast_bf"].T.reshape(2, 128, NPARTS)).astype(f32)
    out["bgT"] = np.ascontiguousarray(inp["ast_bg"].T.reshape(2, 128, NPARTS)).astype(f32)
    bkp = np.stack([inp["ast_bk"][i] + inp["ast_wk"][i] @ inp["ast_bh"][i]
                    for i in range(NPARTS)])
    out["bkpT"] = np.ascontiguousarray(bkp.T.reshape(2, 128, NPARTS)).astype(f32)
    out["invLp"] = np.tile(np.array([1.0 / lp for lp in LP], f32)[None, :], (128, 1))
    out["ones"] = np.ones((128, 1), f32)
    out["onesT"] = np.ones((1, 128), f32)
    return out


def prepare_data(inp, ns_total=N):
    """Per-sample x (permuted) and s (part-packed) in (n, 2, 128, T, V) layout."""
    f32 = np.float32
    xp = np.ascontiguousarray(inp["x"][:, :, :, PERM]).astype(f32)
    x = xp.reshape(ns_total, 2, 128, T, V)
    s = np.concatenate(
        [np.asarray(inp[nm])[:, :, :, IDX[i]].transpose(0, 1, 3, 2).reshape(
            ns_total, C, -1) for i, nm in enumerate(
            ["s_leftleg", "s_rightleg", "s_spine", "s_leftarm", "s_rightarm"])],
        axis=2).astype(f32).reshape(ns_total, 2, 128, L)
    return x, s


_CACHE = {}


def _get_nc(ns):
    if ns not in _CACHE:
        _CACHE[ns] = build_kernel(ns)
    return _CACHE[ns]


def run(inputs, trace=False, trace_kwargs=None):
    ns = N // NCORES
    nc = _get_nc(ns)
    w = prepare_weights({k: np.asarray(v) for k, v in inputs.items()})
    x, s = prepare_data({k: np.asarray(v) for k, v in inputs.items()})
    in_maps = []
    for c in range(NCORES):
        m = dict(w)
        m["x"] = np.ascontiguousarray(x[c * ns:(c + 1) * ns])
        m["s"] = np.ascontiguousarray(s[c * ns:(c + 1) * ns])
        in_maps.append(m)
    res = run_bass_kernel_spmd(nc, in_maps, core_ids=list(range(NCORES)),
                               trace=trace, **(trace_kwargs or {}))
    ys = np.concatenate([res.results[c]["y"].reshape(ns, C, T, V)
                         for c in range(NCORES)])
    y = np.empty_like(ys)
    y[:, :, :, PERM] = ys
    return y, res


def kernel(**inputs):
    y, _ = run(inputs)
    return y


# revision 19
# speedup vs baseline: 1.1608x; 1.1608x over previous
"""Trainium2 Bass kernel for nn_Decoder (gnn_message_passing).

Data-parallel over batch N=64 across 8 NeuronCores (8 samples/core).

Per-sample layout: channels c on partitions (2 chunks of 128), l = (t, v)
with v innermost, v permuted part-packed:
  PERM = [1,2,3,4, 5,6,7,8, 0,9,10,11,12, 13..16, 17..20]
so each body part is one contiguous v-range. The adjacency A is permuted
on the host to match; the final output is un-permuted on the host.

Pipeline per sample (all heavy matmuls in fp32r = s8e11m rounded operands,
fp32 accumulate):
  A) AdaIN: bn_stats for x/s parts, tiny MLP (Prelu alpha=0.2), fused
     affine+leaky-relu apply -> x1 (c, l) f32r
  B) out1T (l-chunk 126, 768) = x1^T @ Wall^T   [stationary = x1 chunks]
  C) out2T (l-chunk, c) = sum_k Bd_k^T-style matmul [stationary = kron(I6, A_k)]
     then PE-transpose back to (c, l-chunk), + folded gcn bias -> out2p (padded t)
  D) tcn: 36 matmuls over shifted windows of out2p, + tcn bias -> x2 (c, l)
  E) astyle per part: inorm (fused in ACT), F/G convs, HmT via stationary=s,
     S_pre^T via stationary=Gm, exp (no max-sub needed; values bounded),
     softmax denominator via ones-matmul, O = HmT.T @ E^T, wk conv,
     deferred 1/D scaling, +bk' (bh folded via softmax-sum=1), residual -> x3
  F) = B/C/D with layer-2 weights -> y
"""
import numpy as np

import concourse.bacc as bacc
import concourse.tile as tile
from concourse import mybir
from concourse.bass_utils import run_bass_kernel_spmd
from concourse.masks import make_identity

F32 = mybir.dt.float32
F32R = mybir.dt.float32r

N, C, T, V, K, KT, LAT = 64, 256, 60, 21, 3, 3, 64
IDX = [[1, 2, 3, 4], [5, 6, 7, 8], [0, 9, 10, 11, 12], [13, 14, 15, 16], [17, 18, 19, 20]]
PERM = np.array(sum(IDX, []))
PART_OFF = [0, 4, 8, 13, 17, 21]
NPARTS = 5
NCORES = 8
L = T * V               # 1260
NL = 10                 # l-chunks
LC = L // NL            # 126 = 6 t-rows
TG = 6                  # t-rows per l-chunk
ALPHA = 0.2
EPS = 1e-5

# l2-chunk sizes per part (aligned to whole t-rows)
PART_LP = [PART_OFF[i + 1] - PART_OFF[i] for i in range(NPARTS)]        # joints
LP = [T * v for v in PART_LP]                                            # 240/300
LPP = [max(256, lp) for lp in LP]                                        # padded N
SOFF = [60 * o for o in PART_OFF]
M_CHUNKS = {240: [(0, 120), (120, 240)], 300: [(0, 100), (100, 200), (200, 300)]}


def build_kernel(ns):
    """Build the Bass program for `ns` samples per core. Returns nc."""
    nc = bacc.Bacc("TRN2", target_bir_lowering=False)

    # ---------------- DRAM tensors ----------------
    d_x = nc.dram_tensor("x", [ns, 2, 128, T, V], F32R, kind="ExternalInput").ap()
    d_s = nc.dram_tensor("s", [ns, 2, 128, L], F32R, kind="ExternalInput").ap()
    d_wall = [nc.dram_tensor(f"wall{ly}", [2, 128, K * C], F32R, kind="ExternalInput").ap()
              for ly in range(2)]
    d_bd = nc.dram_tensor("bd", [K, LC, LC], F32R, kind="ExternalInput").ap()
    d_tcnw = [nc.dram_tensor(f"tcnw{ly}", [KT, 2, 128, C], F32R, kind="ExternalInput").ap()
              for ly in range(2)]
    d_b2v = [nc.dram_tensor(f"b2v{ly}", [2, 128, 1, V], F32, kind="ExternalInput").ap()
             for ly in range(2)]
    d_tcnb = [nc.dram_tensor(f"tcnb{ly}", [2, 128, 1], F32, kind="ExternalInput").ap()
              for ly in range(2)]
    d_wlT = nc.dram_tensor("wlT", [NPARTS, 2, 128, LAT], F32, kind="ExternalInput").ap()
    d_w1T = nc.dram_tensor("w1T", [NPARTS, LAT, LAT], F32, kind="ExternalInput").ap()
    d_w2T = nc.dram_tensor("w2T", [NPARTS, LAT, 2 * C], F32, kind="ExternalInput").ap()
    d_blT = nc.dram_tensor("blT", [LAT, NPARTS], F32, kind="ExternalInput").ap()
    d_b1T = nc.dram_tensor("b1T", [LAT, NPARTS], F32, kind="ExternalInput").ap()
    d_b2q = nc.dram_tensor("b2q", [128, NPARTS, 4, 1], F32, kind="ExternalInput").ap()
    d_astw = {nm: nc.dram_tensor(f"ast_w{nm}", [NPARTS, 2, 128, C], F32R,
                                 kind="ExternalInput").ap() for nm in "fghk"}
    d_bf = nc.dram_tensor("bfT", [2, 128, NPARTS], F32, kind="ExternalInput").ap()
    d_bg = nc.dram_tensor("bgT", [2, 128, NPARTS], F32, kind="ExternalInput").ap()
    d_bkp = nc.dram_tensor("bkpT", [2, 128, NPARTS], F32, kind="ExternalInput").ap()
    d_invLp = nc.dram_tensor("invLp", [128, NPARTS], F32, kind="ExternalInput").ap()
    d_ones = nc.dram_tensor("ones", [128, 128], F32R, kind="ExternalInput").ap()
    d_y = nc.dram_tensor("y", [ns, 2, 128, T, V], F32, kind="ExternalOutput").ap()

    AF = mybir.ActivationFunctionType
    OP = mybir.AluOpType

    with tile.TileContext(nc) as tc:
        with tc.tile_pool(name="consts", bufs=1) as cs, \
             tc.tile_pool(name="stream", bufs=1) as st, \
             tc.tile_pool(name="psum", bufs=8, space="PSUM") as ps:
            _ctr = [0]

            def _mk(pool):
                def f(shape, dtype=F32, tag="t", bufs=None):
                    _ctr[0] += 1
                    kw = {} if bufs is None else {"bufs": bufs}
                    return pool.tile(shape, dtype, tag=tag, name=f"{tag}_{_ctr[0]}", **kw)
                return f
            _cs, _st, _ps = _mk(cs), _mk(st), _mk(ps)
            # ---------------- load constants ----------------
            wall = [[_cs([128, K * C], F32R, tag=f"wall{ly}_{ch}") for ch in range(2)]
                    for ly in range(2)]
            for ly in range(2):
                for ch in range(2):
                    nc.sync.dma_start(out=wall[ly][ch], in_=d_wall[ly][ch])
            bd = [_cs([LC, LC], F32R, tag=f"bd{k}") for k in range(K)]
            for k in range(K):
                nc.sync.dma_start(out=bd[k], in_=d_bd[k])
            tcnw = [[[_cs([128, C], F32R, tag=f"tcnw{ly}_{dt}_{ch}") for ch in range(2)]
                     for dt in range(KT)] for ly in range(2)]
            for ly in range(2):
                for dt in range(KT):
                    for ch in range(2):
                        nc.sync.dma_start(out=tcnw[ly][dt][ch], in_=d_tcnw[ly][dt][ch])
            b2v = [[_cs([128, 1, V], F32, tag=f"b2v{ly}_{ch}") for ch in range(2)]
                   for ly in range(2)]
            tcnb = [[_cs([128, 1], F32, tag=f"tcnb{ly}_{ch}") for ch in range(2)]
                    for ly in range(2)]
            for ly in range(2):
                for ch in range(2):
                    nc.sync.dma_start(out=b2v[ly][ch], in_=d_b2v[ly][ch])
                    nc.sync.dma_start(out=tcnb[ly][ch], in_=d_tcnb[ly][ch])
            wlT = [[_cs([128, LAT], F32, tag=f"wlT{i}_{ch}") for ch in range(2)]
                   for i in range(NPARTS)]
            w1T = [_cs([LAT, LAT], F32, tag=f"w1T{i}") for i in range(NPARTS)]
            w2T = [_cs([LAT, 2 * C], F32, tag=f"w2T{i}") for i in range(NPARTS)]
            for i in range(NPARTS):
                for ch in range(2):
                    nc.sync.dma_start(out=wlT[i][ch], in_=d_wlT[i][ch])
                nc.sync.dma_start(out=w1T[i], in_=d_w1T[i])
                nc.sync.dma_start(out=w2T[i], in_=d_w2T[i])
            blT = _cs([LAT, NPARTS], F32, tag="blT")
            b1T = _cs([LAT, NPARTS], F32, tag="b1T")
            b2q = _cs([128, NPARTS, 4, 1], F32, tag="b2q")
            nc.sync.dma_start(out=blT, in_=d_blT)
            nc.sync.dma_start(out=b1T, in_=d_b1T)
            nc.sync.dma_start(out=b2q, in_=d_b2q)
            astw = {nm: [[_cs([128, C], F32R, tag=f"astw{nm}{i}_{ch}")
                          for ch in range(2)] for i in range(NPARTS)] for nm in "fghk"}
            for nm in "fghk":
                for i in range(NPARTS):
                    for ch in range(2):
                        nc.sync.dma_start(out=astw[nm][i][ch], in_=d_astw[nm][i][ch])
            bf = [_cs([128, NPARTS], F32, tag=f"bf{ch}") for ch in range(2)]
            bg = [_cs([128, NPARTS], F32, tag=f"bg{ch}") for ch in range(2)]
            bkp = [_cs([128, NPARTS], F32, tag=f"bkp{ch}") for ch in range(2)]
            for ch in range(2):
                nc.sync.dma_start(out=bf[ch], in_=d_bf[ch])
                nc.sync.dma_start(out=bg[ch], in_=d_bg[ch])
                nc.sync.dma_start(out=bkp[ch], in_=d_bkp[ch])
            ident = _cs([128, 128], F32, tag="ident")
            make_identity(nc, ident)
            onesr = _cs([128, 128], F32R, tag="onesr")
            nc.sync.dma_start(out=onesr, in_=d_ones)
            epsT = _cs([128, 1], F32, tag="epsT")
            nc.vector.memset(epsT, EPS)
            invLp = _cs([128, NPARTS], F32, tag="invLp")
            nc.sync.dma_start(out=invLp, in_=d_invLp)

            def part_stats(get_ap, mtag):
                mean = [_st([128, NPARTS], F32, tag=f"{mtag}m_{ch}", bufs=2)
                        for ch in range(2)]
                var = [_st([128, NPARTS], F32, tag=f"{mtag}v_{ch}", bufs=2)
                       for ch in range(2)]
                for ch in range(2):
                    sums = _st([128, NPARTS], F32, tag="stsum")
                    sqs = _st([128, NPARTS], F32, tag="stsq")
                    for i in range(NPARTS):
                        lo, hi = PART_OFF[i], PART_OFF[i + 1]
                        src_ap = get_ap(ch, i)
                        ax = (mybir.AxisListType.X if len(src_ap.shape) == 2
                              else mybir.AxisListType.XY)
                        nc.vector.reduce_sum(out=sums[:, i:i + 1], in_=src_ap,
                                             axis=ax)
                        sc_ = _st([128, T * 5], F32, tag="sqscr", bufs=2)
                        sc_ap = sc_[:, :LP[i]]
                        if len(src_ap.shape) == 3:
                            sc_ap = sc_ap.rearrange("p (a b) -> p a b",
                                                    b=src_ap.shape[2])
                        nc.scalar.activation(out=sc_ap, in_=src_ap,
                                             func=AF.Square,
                                             accum_out=sqs[:, i:i + 1])
                    nc.vector.tensor_mul(mean[ch], sums, invLp)
                    msq = _st([128, NPARTS], F32, tag="stmsq")
                    nc.vector.tensor_mul(msq, sqs, invLp)
                    m2 = _st([128, NPARTS], F32, tag="stm2")
                    nc.vector.tensor_mul(m2, mean[ch], mean[ch])
                    nc.vector.tensor_sub(var[ch], msq, m2)
                return mean, var

            def mm(out, pairs, **kw):
                nmm = len(pairs)
                for j, (lh, rh) in enumerate(pairs):
                    nc.tensor.matmul(out, lh, rh, start=(j == 0), stop=(j == nmm - 1), **kw)

            # ---------------- gcn + tcn block ----------------
            def gcn_tcn(ly, xin, xout_dtype, bias_out_tag):
                """xin: [2] tiles (128, T, V) f32r. Returns x2 [2] tiles (128,T,V)."""
                out2p = [_st([128, T + 2, V], F32R, tag=f"out2p_{ch}") for ch in range(2)]
                for g in range(NL):
                    psB = [_ps([LC, 384], F32, tag="mm") for _ in range(2)]
                    for nsl in range(2):
                        mm(psB[nsl],
                           [(xin[ch][:, g * TG:(g + 1) * TG, :],
                             wall[ly][ch][:, nsl * 384:(nsl + 1) * 384]) for ch in range(2)])
                    o1 = _st([LC, K * C], F32R, tag="out1T", bufs=2)
                    for nsl in range(2):
                        nc.vector.tensor_copy(o1[:, nsl * 384:(nsl + 1) * 384], psB[nsl])
                    psC = _ps([LC, C], F32, tag="mm")
                    mm(psC, [(bd[k], o1[:, k * C:(k + 1) * C]) for k in range(K)])
                    o2 = _st([LC, C], F32, tag="out2T", bufs=2)
                    nc.vector.tensor_copy(o2, psC)
                    for ch in range(2):
                        psT = _ps([128, LC], F32, tag="mm")
                        nc.tensor.transpose(psT, o2[:, ch * 128:(ch + 1) * 128],
                                            ident[:LC, :LC])
                        # evict + folded gcn bias -> padded rows [1..61)
                        nc.vector.tensor_tensor(
                            out=out2p[ch][:, g * TG + 1:(g + 1) * TG + 1, :],
                            in0=psT.rearrange("p (a b) -> p a b", b=V),
                            in1=b2v[ly][ch].to_broadcast([128, TG, V]),
                            op=OP.add)
                # reflect pads: t'=0 <- t'=2 ; t'=61 <- t'=59
                for ch in range(2):
                    nc.vector.tensor_copy(out2p[ch][:, 0, :], out2p[ch][:, 2, :])
                    nc.vector.tensor_copy(out2p[ch][:, T + 1, :], out2p[ch][:, T - 1, :])
                # tcn
                x2 = [_st([128, T, V], xout_dtype, tag=f"{bias_out_tag}_{ch}",
                           bufs=(2 if bias_out_tag == "y" else 1))
                      for ch in range(2)]
                for cch in range(2):
                    flat = [out2p[ch].rearrange("p a b -> p (a b)") for ch in range(2)]
                    for nsl in range(3):
                        psD = _ps([128, 420], F32, tag="mm")
                        mm(psD, [(tcnw[ly][dt][cin][:, cch * 128:(cch + 1) * 128],
                                  flat[cin][:, dt * V + nsl * 420:dt * V + nsl * 420 + 420])
                                 for cin in range(2) for dt in range(KT)])
                        nc.scalar.activation(
                            out=x2[cch].rearrange("p a b -> p (a b)")[:, nsl * 420:(nsl + 1) * 420],
                            in_=psD, func=AF.Identity, bias=tcnb[ly][cch], scale=1.0)
                return x2

            # ---------------- pre-pass: s stats for all samples ----------------
            sbar_all = [_st([128, NPARTS, ns], F32R, tag=f"sbarall_{ch}")
                        for ch in range(2)]
            svar_all = [_st([128, NPARTS, ns], F32, tag=f"svarall_{ch}")
                        for ch in range(2)]
            for n in range(ns):
                s_pre = [_st([128, L], F32R, tag=f"s_{ch}") for ch in range(2)]
                for ch in range(2):
                    nc.sync.dma_start(out=s_pre[ch], in_=d_s[n, ch])
                    sums = _st([128, NPARTS], F32, tag="stsum")
                    sqs = _st([128, NPARTS], F32, tag="stsq")
                    for i in range(NPARTS):
                        src_ap = s_pre[ch][:, SOFF[i]:SOFF[i + 1]].bitcast(F32)
                        nc.vector.reduce_sum(out=sums[:, i:i + 1], in_=src_ap,
                                             axis=mybir.AxisListType.X)
                        sc_ = _st([128, T * 5], F32, tag="sqscr", bufs=2)
                        nc.scalar.activation(out=sc_[:, :LP[i]], in_=src_ap,
                                             func=AF.Square,
                                             accum_out=sqs[:, i:i + 1])
                    with nc.allow_low_precision(reason="adain mlp operand"):
                        nc.vector.tensor_mul(sbar_all[ch][:, :, n], sums, invLp)
                    msq = _st([128, NPARTS], F32, tag="stmsq")
                    nc.vector.tensor_mul(msq, sqs, invLp)
                    m2 = _st([128, NPARTS], F32, tag="stm2")
                    nc.vector.tensor_mul(m2, sbar_all[ch][:, :, n].bitcast(F32),
                                         sbar_all[ch][:, :, n].bitcast(F32))
                    nc.vector.tensor_sub(svar_all[ch][:, :, n], msq, m2)

            # ---------------- batched AdaIN MLP (N = ns) ----------------
            gbe_big = _st([128, NPARTS, 4, ns], F32, tag="gbe_big")
            for i in range(NPARTS):
                psZ = _ps([LAT, ns], F32, tag="mm")
                mm(psZ, [(wlT[i][ch], sbar_all[ch][:, i, :].bitcast(F32)) for ch in range(2)])
                z = _st([LAT, ns], F32, tag="z")
                nc.scalar.activation(out=z, in_=psZ, func=AF.Prelu,
                                     bias=blT[:, i:i + 1], scale=1.0, alpha=ALPHA)
                psH = _ps([LAT, ns], F32, tag="mm")
                mm(psH, [(w1T[i], z)])
                h = _st([LAT, ns], F32, tag="z")
                nc.scalar.activation(out=h, in_=psH, func=AF.Prelu,
                                     bias=b1T[:, i:i + 1], scale=1.0, alpha=ALPHA)
                psG = _ps([128, 4, ns], F32, tag="mm")
                for j in range(4):
                    nc.tensor.matmul(psG[:, j, :], w2T[i][:, j * 128:(j + 1) * 128],
                                     h, start=True, stop=True)
                nc.vector.tensor_tensor(out=gbe_big[:, i, :, :], in0=psG,
                                        in1=b2q[:, i].to_broadcast([128, 4, ns]),
                                        op=OP.add)

            # ---------------- per-sample ----------------
            for n in range(ns):
                x_sb = [_st([128, T, V], F32R, tag=f"x_{ch}", bufs=2) for ch in range(2)]
                s_sb = [_st([128, L], F32R, tag=f"s_{ch}") for ch in range(2)]
                for ch in range(2):
                    nc.sync.dma_start(out=x_sb[ch], in_=d_x[n, ch])
                    nc.sync.dma_start(out=s_sb[ch], in_=d_s[n, ch])

                # ---- Phase A: x stats ----
                xmean, xvar = part_stats(
                    lambda ch, i: x_sb[ch][:, :, PART_OFF[i]:PART_OFF[i + 1]].bitcast(F32),
                    "mvx")
                smean = [sbar_all[ch][:, :, n].bitcast(F32) for ch in range(2)]
                svar = [svar_all[ch][:, :, n] for ch in range(2)]

                # ---- Phase A: massage + apply (in place: x_sb becomes x1) ----
                x1 = x_sb
                for ch in range(2):
                    sq = _st([128, NPARTS], F32, tag="sq")
                    nc.scalar.activation(out=sq, in_=xvar[ch], func=AF.Sqrt,
                                         bias=epsT, scale=1.0)
                    rstd = _st([128, NPARTS], F32, tag="rstd")
                    nc.vector.reciprocal(out=rstd, in_=sq)
                    scale = _st([128, NPARTS], F32, tag="scl")
                    nc.vector.scalar_tensor_tensor(out=scale, in0=gbe_big[:, :, ch, n],
                                                   scalar=1.0, in1=rstd,
                                                   op0=OP.add, op1=OP.mult)
                    tmp = _st([128, NPARTS], F32, tag="tmp5")
                    nc.vector.tensor_mul(tmp, xmean[ch], scale)
                    shift = _st([128, NPARTS], F32, tag="shf")
                    nc.vector.scalar_tensor_tensor(out=shift, in0=tmp, scalar=-1.0,
                                                   in1=gbe_big[:, :, 2 + ch, n],
                                                   op0=OP.mult, op1=OP.add)
                    for i in range(NPARTS):
                        lo, hi = PART_OFF[i], PART_OFF[i + 1]
                        nc.scalar.activation(out=x1[ch][:, :, lo:hi],
                                             in_=x_sb[ch][:, :, lo:hi].bitcast(F32),
                                             func=AF.Prelu,
                                             bias=shift[:, i:i + 1],
                                             scale=scale[:, i:i + 1], alpha=ALPHA)

                # ---- Phase B+C+D: layer 1 ----
                x2 = gcn_tcn(0, x1, F32, "x2")

                # ---- Phase E: astyle ----
                # x2 part stats
                mean2, var2 = part_stats(
                    lambda ch, i: x2[ch][:, :, PART_OFF[i]:PART_OFF[i + 1]], "mv2")
                # massage: rstd/shift for x2 and s
                rs2, sh2, rss, shs = [], [], [], []
                for ch in range(2):
                    for (mn, vr, rs_l, sh_l, t_) in (
                            (mean2[ch], var2[ch], rs2, sh2, "i2"),
                            (smean[ch], svar[ch], rss, shs, "is")):
                        sq = _st([128, NPARTS], F32, tag="sq")
                        nc.scalar.activation(out=sq, in_=vr, func=AF.Sqrt,
                                             bias=epsT, scale=1.0)
                        rstd = _st([128, NPARTS], F32, tag=f"rstd_{t_}", bufs=2)
                        nc.vector.reciprocal(out=rstd, in_=sq)
                        shift = _st([128, NPARTS], F32, tag=f"shf_{t_}", bufs=2)
                        tmp = _st([128, NPARTS], F32, tag="tmp5")
                        nc.vector.tensor_mul(tmp, mn, rstd)
                        nc.vector.tensor_scalar_mul(shift, tmp, -1.0)
                        rs_l.append(rstd)
                        sh_l.append(shift)

                x3 = [_st([128, T, V], F32R, tag=f"x3_{ch}") for ch in range(2)]
                for i in range(NPARTS):
                    lo, hi = PART_OFF[i], PART_OFF[i + 1]
                    lp, lpp = LP[i], LPP[i]
                    mchunks = M_CHUNKS[lp]
                    inx = [_st([128, lpp], F32R, tag=f"inx_{ch}") for ch in range(2)]
                    ins = [_st([128, lpp], F32R, tag=f"ins_{ch}") for ch in range(2)]
                    for ch in range(2):
                        nc.scalar.activation(
                            out=inx[ch][:, :lp].rearrange("p (a b) -> p a b", b=T),
                            in_=x2[ch][:, :, lo:hi].rearrange("p a b -> p b a"),
                            func=AF.Identity,
                            bias=sh2[ch][:, i:i + 1], scale=rs2[ch][:, i:i + 1])
                        nc.scalar.activation(
                            out=ins[ch][:, :lp],
                            in_=s_sb[ch][:, SOFF[i]:SOFF[i + 1]].bitcast(F32),
                            func=AF.Identity,
                            bias=shs[ch][:, i:i + 1], scale=rss[ch][:, i:i + 1])
                        if lp < lpp:
                            for padt in (inx[ch], ins[ch]):
                                nc.scalar.activation(
                                    out=padt[:, lp:], in_=ident[:, :lpp - lp],
                                    func=AF.Identity, scale=0.0, bias=0.0)
                    Fm, Gm = [], []
                    for (wt, src, bias_t, dst) in (("f", inx, bf, Fm), ("g", ins, bg, Gm)):
                        for cout in range(2):
                            psF = _ps([128, lpp], F32, tag="mm")
                            mm(psF, [(astw[wt][i][cin][:, cout * 128:(cout + 1) * 128],
                                      src[cin]) for cin in range(2)])
                            o = _st([128, lpp], F32R, tag=f"{wt}m_{cout}")
                            nc.scalar.activation(out=o, in_=psF, func=AF.Identity,
                                                 bias=bias_t[cout][:, i:i + 1], scale=1.0)
                            dst.append(o)
                    HmT = []
                    for (mlo, mhi) in mchunks:
                        sz = mhi - mlo
                        psHm = _ps([sz, C], F32, tag="mm")
                        mm(psHm, [(s_sb[cin][:, SOFF[i] + mlo:SOFF[i] + mhi],
                                   astw["h"][i][cin]) for cin in range(2)])
                        o = _st([128, C], F32R, tag="HmT", bufs=3)
                        nc.vector.tensor_copy(o[:sz], psHm)
                        HmT.append(o)
                    Et = []
                    for mi_, (mlo, mhi) in enumerate(mchunks):
                        sz = mhi - mlo
                        psS = _ps([sz, lpp], F32, tag="mm")
                        mm(psS, [(Gm[cch][:, mlo:mhi], Fm[cch]) for cch in range(2)])
                        o = _st([128, lpp], F32R, tag="Et", bufs=3)
                        nc.scalar.activation(out=o[:sz], in_=psS, func=AF.Exp, scale=1.0)
                        Et.append(o)
                    psDb = _ps([128, lp], F32, tag="mm")
                    mm(psDb, [(onesr[:mhi - mlo, :], Et[j][:mhi - mlo, :lp])
                              for j, (mlo, mhi) in enumerate(mchunks)])
                    Db = _st([128, lp], F32, tag="Db")
                    nc.vector.reciprocal(out=Db, in_=psDb)
                    O_sb = []
                    for cout in range(2):
                        psO = _ps([128, lpp], F32, tag="mm")
                        mm(psO, [(HmT[j][:mhi - mlo, cout * 128:(cout + 1) * 128],
                                  Et[j][:mhi - mlo])
                                 for j, (mlo, mhi) in enumerate(mchunks)])
                        o = _st([128, lpp], F32R, tag=f"O_{cout}")
                        nc.vector.tensor_copy(o, psO)
                        O_sb.append(o)
                    for cout in range(2):
                        psK = _ps([128, lpp], F32, tag="mm")
                        mm(psK, [(astw["k"][i][cin][:, cout * 128:(cout + 1) * 128],
                                  O_sb[cin]) for cin in range(2)])
                        t1_ = _st([128, hi - lo, T], F32, tag="wkt")
                        nc.vector.tensor_tensor(
                            out=t1_,
                            in0=psK[:, :lp].rearrange("p (a b) -> p a b", b=T),
                            in1=Db.rearrange("p (a b) -> p a b", b=T),
                            op=OP.mult)
                        nc.vector.scalar_tensor_tensor(
                            out=x3[cout][:, :, lo:hi].rearrange("p a b -> p b a"),
                            in0=t1_, scalar=bkp[cout][:, i:i + 1],
                            in1=x2[cout][:, :, lo:hi].rearrange("p a b -> p b a"),
                            op0=OP.add, op1=OP.add)

                # ---- Phase F: layer 2 ----
                y_sb = gcn_tcn(1, x3, F32, "y")
                for ch in range(2):
                    nc.sync.dma_start(out=d_y[n, ch], in_=y_sb[ch])

    nc.compile()
    return nc


def prepare_weights(inp):
    """Host-side weight preprocessing -> dict of DRAM arrays (shared across cores)."""
    f32 = np.float32
    Ap = np.ascontiguousarray(inp["A"][:, PERM][:, :, PERM]).astype(f32)
    out = {}
    out["bd"] = np.stack([np.kron(np.eye(TG, dtype=f32), Ap[k]) for k in range(K)])
    for ly, (gw, gb, tw, tb) in enumerate(
            ((inp["gcn1_w"], inp["gcn1_b"], inp["tcn1_w"], inp["tcn1_b"]),
             (inp["gcn2_w"], inp["gcn2_b"], inp["tcn2_w"], inp["tcn2_b"]))):
        out[f"wall{ly}"] = np.ascontiguousarray(gw.T.reshape(2, 128, K * C)).astype(f32)
        out[f"tcnw{ly}"] = np.ascontiguousarray(
            tw[:, :, :, 0].transpose(2, 1, 0).reshape(KT, 2, 128, C)).astype(f32)
        bias2 = np.einsum("kc,kw->cw", gb.reshape(K, C), Ap.sum(axis=1)).astype(f32)
        out[f"b2v{ly}"] = np.ascontiguousarray(
            bias2.reshape(2, 128, 1, V)).astype(f32)
        out[f"tcnb{ly}"] = np.ascontiguousarray(tb.reshape(2, 128, 1)).astype(f32)
    out["wlT"] = np.ascontiguousarray(
        inp["adain_wl"].transpose(0, 2, 1).reshape(NPARTS, 2, 128, LAT)).astype(f32)
    out["w1T"] = np.ascontiguousarray(inp["adain_w1"].transpose(0, 2, 1)).astype(f32)
    out["w2T"] = np.ascontiguousarray(inp["adain_w2"].transpose(0, 2, 1)).astype(f32)
    out["blT"] = np.ascontiguousarray(inp["adain_bl"].T).astype(f32)
    out["b1T"] = np.ascontiguousarray(inp["adain_b1"].T).astype(f32)
    out["b2q"] = np.ascontiguousarray(
        inp["adain_b2"].reshape(NPARTS, 4, 128).transpose(2, 0, 1)
    ).astype(f32).reshape(128, NPARTS, 4, 1)
    for nm in "fgh":
        out[f"ast_w{nm}"] = np.ascontiguousarray(
            inp[f"ast_w{nm}"].transpose(0, 2, 1).reshape(NPARTS, 2, 128, C)).astype(f32)
    out["ast_wk"] = np.ascontiguousarray(
        inp["ast_wk"].transpose(0, 2, 1).reshape(NPARTS, 2, 128, C)).astype(f32)
    out["bfT"] = np.ascontiguousarray(inp["ast_bf"].T.reshape(2, 128, NPARTS)).astype(f32)
    out["bgT"] = np.ascontiguousarray(inp["ast_bg"].T.reshape(2, 128, NPARTS)).astype(f32)
    bkp = np.stack([inp["ast_bk"][i] + inp["ast_wk"][i] @ inp["ast_bh"][i]
                    for i in range(NPARTS)])
    out["bkpT"] = np.ascontiguousarray(bkp.T.reshape(2, 128, NPARTS)).astype(f32)
    out["invLp"] = np.tile(np.array([1.0 / lp for lp in LP], f32)[None, :], (128, 1))
    out["ones"] = np.ones((128, 128), f32)
    return out


def prepare_data(inp, ns_total=N):
    """Per-sample x (permuted) and s (part-packed) in (n, 2, 128, T, V) layout."""
    f32 = np.float32
    xp = np.ascontiguousarray(inp["x"][:, :, :, PERM]).astype(f32)
    x = xp.reshape(ns_total, 2, 128, T, V)
    s = np.concatenate(
        [np.asarray(inp[nm])[:, :, :, IDX[i]].transpose(0, 1, 3, 2).reshape(
            ns_total, C, -1) for i, nm in enumerate(
            ["s_leftleg", "s_rightleg", "s_spine", "s_leftarm", "s_rightarm"])],
        axis=2).astype(f32).reshape(ns_total, 2, 128, L)
    return x, s


_CACHE = {}


def _get_nc(ns):
    if ns not in _CACHE:
        _CACHE[ns] = build_kernel(ns)
    return _CACHE[ns]


def run(inputs, trace=False, trace_kwargs=None):
    ns = N // NCORES
    nc = _get_nc(ns)
    w = prepare_weights({k: np.asarray(v) for k, v in inputs.items()})
    x, s = prepare_data({k: np.asarray(v) for k, v in inputs.items()})
    in_maps = []
    for c in range(NCORES):
        m = dict(w)
        m["x"] = np.ascontiguousarray(x[c * ns:(c + 1) * ns])
        m["s"] = np.ascontiguousarray(s[c * ns:(c + 1) * ns])
        in_maps.append(m)
    res = run_bass_kernel_spmd(nc, in_maps, core_ids=list(range(NCORES)),
                               trace=trace, **(trace_kwargs or {}))
    ys = np.concatenate([res.results[c]["y"].reshape(ns, C, T, V)
                         for c in range(NCORES)])
    y = np.empty_like(ys)
    y[:, :, :, PERM] = ys
    return y, res


def kernel(**inputs):
    y, _ = run(inputs)
    return y
